# revision 1
# baseline (speedup 1.0000x reference)
"""Attention-LSTM (CaptioningRNN) Trainium2 kernel.

Strategy: data-parallel over the batch N=128 across 8 NeuronCores (16
samples/core), zero cross-core communication.  Per core:

  Phase 1:  xW = x_local @ Wx + b  (fp16 matmuls, full 128-row PE tiles)
            -> fp16 DRAM scratch, rows ordered (t, n);
            c0 = h0 = mean_m(A_local)   (DVE reduce).
  Phase 2:  64 sequential LSTM steps.  The two recurrent GEMMs
            (h @ Wh, attn @ Wattn) run as bf16 matmuls: transposed state
            chunks are the stationary operand, SBUF-resident bf16 weights
            are the moving operand; xW_t is folded into the same PSUM
            accumulation via an identity matmul.  Attention:
              scores  = hT.T @ A_chunks (bf16 PE) -> diagonal extraction
                        via mask-multiply + reduce (DVE);
              softmax = ACT exp + DVE reduce/reciprocal;
              attn^T  = softmax weights broadcast across partitions with
                        a ones-matmul, then DVE multiply + reduce,
                        produced directly in the transposed layout the
                        next GEMM needs.
"""

import sys

sys.path.insert(0, "/opt/trn_rl_repo")

import ml_dtypes
import numpy as np

import concourse.bass as bass  # noqa: F401
import concourse.mybir as mybir
import concourse.tile as tile
from concourse import bacc
from concourse.bass_utils import run_bass_kernel_spmd

F32 = mybir.dt.float32
F32R = mybir.dt.float32r
BF16 = mybir.dt.float16  # IEEE fp16: same PE rate as bf16, 4x the mantissa precision

N, T, D, H = 128, 64, 1024, 1024
K4 = 4 * H            # 4096
NCORES = 8
NL = N // NCORES      # 16 samples per core
M = 16                # spatial positions (4x4)
HC = H // 128         # 8 h-chunks
SCALE = 1.0 / float(np.sqrt(H))

_cache = {}

_SKIP = set()  # build variants used only during development profiling


def _build(steps=T):
    key = ("nc", steps)
    if key in _cache:
        return _cache[key]

    nc = bacc.Bacc("TRN2", target_bir_lowering=False)

    # ---- kernel I/O ----------------------------------------------------
    # xT rows are D, columns are (t, n) so step t's block is contiguous.
    d_xT = nc.dram_tensor("xT", [D, T * NL], BF16, kind="ExternalInput")
    d_A = nc.dram_tensor("A", [NL, H, M], F32, kind="ExternalInput")
    d_Wx = nc.dram_tensor("Wx", [D, K4], BF16, kind="ExternalInput")
    d_Wh = nc.dram_tensor("Wh", [H, K4], BF16, kind="ExternalInput")
    d_Wa = nc.dram_tensor("Wa", [H, K4], BF16, kind="ExternalInput")
    d_b = nc.dram_tensor("b", [1, K4], BF16, kind="ExternalInput")
    d_id = nc.dram_tensor("ident", [16, 16], F32, kind="ExternalInput")
    d_mmn = nc.dram_tensor("mask_mn", [16, 16 * 16], F32, kind="ExternalInput")
    d_mnm = nc.dram_tensor("mask_nm", [16, 16 * 16], F32, kind="ExternalInput")
    d_ones = nc.dram_tensor("ones16", [16, 128], BF16, kind="ExternalInput")
    d_ssum = nc.dram_tensor("ssum", [128, 16], BF16, kind="ExternalInput")
    d_y = nc.dram_tensor("y", [NL, T, H], F32, kind="ExternalOutput")

    # One scratch tensor per 8-step row block so phase-2 steps only depend
    # on the phase-1 chunk that produced their rows, not all of phase 1.
    d_xws = [
        nc.dram_tensor(f"xw_scratch_{mc}", [8 * NL, K4], BF16) for mc in range(HC)
    ]

    with tile.TileContext(nc) as tc:
      with tc.tile_pool(name="state", bufs=1) as stp:
        c_sb = stp.tile([NL, H], F32, tag="c")

        # ============== Phase 1: xW = x @ Wx + b, c0 ====================
        with (
            tc.tile_pool(name="p1w", bufs=1) as p1w,
            tc.tile_pool(name="p1s", bufs=2) as p1s,
            tc.tile_pool(name="p1p", bufs=4, space="PSUM") as p1p,
        ):
            wx = p1w.tile([128, HC, K4], BF16, tag="wx")
            nc.sync.dma_start(
                out=wx[:], in_=d_Wx.rearrange("(kc p) f -> p kc f", p=128)
            )
            bias = p1w.tile([1, K4], BF16, tag="bias")
            nc.sync.dma_start(out=bias[:], in_=d_b[:])
            ones1 = p1w.tile([1, 128], BF16, tag="ones1")
            nc.vector.memset(ones1[:], 1.0)

            xt = p1w.tile([128, HC, T * NL], BF16, tag="xt")
            nc.sync.dma_start(
                out=xt[:], in_=d_xT.rearrange("(kc p) r -> p kc r", p=128)
            )

            # c0 = mean over m of A  (layout [n, h])
            for hh in range(4):
                hs = H // 4
                a_n = p1s.tile([NL, hs, M], F32, tag="a_n")
                nc.sync.dma_start(
                    out=a_n[:], in_=d_A[:, hh * hs : (hh + 1) * hs, :]
                )
                csum = p1s.tile([NL, hs], F32, tag="csum")
                nc.vector.tensor_reduce(
                    csum[:], a_n[:], axis=mybir.AxisListType.X, op=mybir.AluOpType.add
                )
                nc.scalar.mul(c_sb[:, hh * hs : (hh + 1) * hs], csum[:], 1.0 / M)

            # xW GEMM over 8 row-chunks of (t, n)
            for mc in range(HC):
                for j in range(8):
                    pj = p1p.tile([128, 512], F32, tag="p1psum")
                    for kc in range(HC):
                        nc.tensor.matmul(
                            pj[:],
                            xt[:, kc, mc * 128 : (mc + 1) * 128],
                            wx[:, kc, j * 512 : (j + 1) * 512],
                            start=(kc == 0),
                            stop=False,
                        )
                    nc.tensor.matmul(
                        pj[:],
                        ones1[:],
                        bias[:, j * 512 : (j + 1) * 512],
                        start=False,
                        stop=True,
                    )
                    ob = p1s.tile([128, 512], BF16, tag="p1out")
                    nc.vector.tensor_copy(ob[:], pj[:])
                    nc.sync.dma_start(
                        out=d_xws[mc][:, j * 512 : (j + 1) * 512],
                        in_=ob[:],
                    )

        # ============== Phase 2: recurrent loop =========================
        with (
            tc.tile_pool(name="wts", bufs=1) as wts,
            tc.tile_pool(name="stt", bufs=1) as stt,
            tc.tile_pool(name="xwp", bufs=2) as xwp,
            tc.tile_pool(name="gat", bufs=1) as gat,
            tc.tile_pool(name="sml", bufs=2) as sml,
            tc.tile_pool(name="big", bufs=1) as big,
            tc.tile_pool(name="actp", bufs=2, space="PSUM") as actp,
            tc.tile_pool(name="packp", bufs=2, space="PSUM") as packp,
            tc.tile_pool(name="trp", bufs=2, space="PSUM") as trp,
            tc.tile_pool(name="scp", bufs=1, space="PSUM") as scp,
        ):
            wh = wts.tile([128, HC, K4], BF16, tag="wh")
            nc.sync.dma_start(out=wh[:], in_=d_Wh.rearrange("(kc p) f -> p kc f", p=128))
            wa = wts.tile([128, HC, K4], BF16, tag="wa")
            nc.sync.dma_start(out=wa[:], in_=d_Wa.rearrange("(kc p) f -> p kc f", p=128))
            # A[n, hc*128+p, m] in transposed per-chunk layout, bf16
            at4 = wts.tile([128, HC, NL, M], BF16, tag="at4")
            for hc in range(HC):
                a_stage = sml.tile([128, NL, M], F32, tag="a_stage")
                nc.sync.dma_start(
                    out=a_stage[:],
                    in_=d_A.rearrange("n (hc p) m -> hc p n m", hc=HC)[hc],
                )
                nc.vector.tensor_copy(at4[:, hc], a_stage[:])
            id16 = wts.tile([16, 16], F32, tag="id16")
            nc.sync.dma_start(out=id16[:], in_=d_id[:])
            mask_mn = wts.tile([16, M, NL], F32, tag="mask_mn")
            nc.sync.dma_start(
                out=mask_mn[:], in_=d_mmn.rearrange("p (a c) -> p a c", a=M)
            )
            mask_nm = wts.tile([16, NL, M], F32, tag="mask_nm")
            nc.sync.dma_start(
                out=mask_nm[:], in_=d_mnm.rearrange("p (a c) -> p a c", a=NL)
            )
            ones16 = wts.tile([16, 128], BF16, tag="ones16")
            nc.sync.dma_start(out=ones16[:], in_=d_ones[:])
            id16b = wts.tile([16, 16], BF16, tag="id16b")
            nc.vector.tensor_copy(id16b[:], id16[:])
            ssum = wts.tile([128, 16], BF16, tag="ssum")
            nc.sync.dma_start(out=ssum[:], in_=d_ssum[:])

            h_sb = stt.tile([NL, H], F32, tag="h")
            nc.vector.tensor_copy(h_sb[:], c_sb[:])

            hT_b = stt.tile([128, HC, NL], BF16, tag="hT_b")
            aT_b = stt.tile([128, HC, NL], BF16, tag="aT_b")

            def transposes(src_sb):
                for k in range(HC):
                    pt = trp.tile([128, NL], F32, tag="trps")
                    nc.tensor.transpose(
                        pt[:], src_sb[:, k * 128 : (k + 1) * 128], id16[:]
                    )
                    nc.vector.tensor_copy(hT_b[:, k], pt[:])

            hpool = stt  # persistent slots for the 8 h-partial tiles

            def emit_h_rounds():
                tiles = []
                for j in range(8):
                    jc = slice(j * 512, (j + 1) * 512)
                    pp = packp.tile([128, 512], F32, tag="pack_ps")
                    for r in range(2):
                        for g in range(4):
                            kk = 4 * r + g
                            nc.tensor.matmul(
                                pp[32 * g : 32 * g + 16, :], hT_b[:, kk],
                                wh[:, kk, jc],
                                start=(r == 0), stop=(r == 1),
                                tile_position=(0, 32 * g),
                                skip_group_check=True,
                            )
                    ph = hpool.tile([128, 512], BF16, tag=f"pps_h{j}")
                    nc.vector.tensor_copy(ph[:], pp[:])
                    tiles.append(ph)
                return tiles

            def attention_scores():
                # scores: S[n, (m, n')] accumulated over h-chunks (bf16 PE)
                ps = scp.tile([16, M * NL], F32, tag="sc_ps")
                for k in range(HC):
                    nc.tensor.matmul(
                        ps[:],
                        hT_b[:, k],
                        at4[:, k].rearrange("p n m -> p m n"),
                        start=(k == 0),
                        stop=(k == HC - 1),
                    )
                smul = sml.tile([16, M, NL], F32, tag="smul")
                nc.vector.tensor_mul(
                    smul[:], ps[:].rearrange("p (m n) -> p m n", m=M), mask_mn[:]
                )
                sc = sml.tile([16, M], F32, tag="sc")
                nc.vector.tensor_reduce(
                    sc[:], smul[:], axis=mybir.AxisListType.X, op=mybir.AluOpType.add
                )
                # softmax (1/sqrt(H) scale folded into exp)
                mx = sml.tile([16, 1], F32, tag="mx")
                nc.vector.tensor_reduce(
                    mx[:], sc[:], axis=mybir.AxisListType.X, op=mybir.AluOpType.max
                )
                nb = sml.tile([16, 1], F32, tag="nb")
                nc.vector.tensor_scalar_mul(nb[:], mx[:], -SCALE)
                ex = sml.tile([16, M], F32, tag="ex")
                nc.scalar.activation(
                    ex[:], sc[:], mybir.ActivationFunctionType.Exp,
                    bias=nb[:], scale=SCALE,
                )
                sm = sml.tile([16, 1], F32, tag="sm")
                nc.vector.tensor_reduce(
                    sm[:], ex[:], axis=mybir.AxisListType.X, op=mybir.AluOpType.add
                )
                rc = sml.tile([16, 1], F32, tag="rc")
                nc.vector.reciprocal(rc[:], sm[:])
                w16 = sml.tile([16, M], F32, tag="w16")
                nc.vector.tensor_scalar_mul(w16[:], ex[:], rc[:])
                # wB[p, (n, m)] = w[n, m] on every partition p
                wd = sml.tile([16, NL, M], BF16, tag="wd")
                nc.vector.tensor_mul(
                    wd[:],
                    w16[:].unsqueeze(1).broadcast_to([16, NL, M]),
                    mask_nm[:],
                )
                return wd

            def attention_apply(wd):
                pwb = scp.tile([128, NL * M], F32, tag="wb_ps")
                nc.tensor.matmul(
                    pwb[:],
                    ones16[:],
                    wd[:].rearrange("p n m -> p (n m)"),
                    start=True,
                    stop=True,
                )
                wbs = sml.tile([128, NL, M], BF16, tag="wbs")
                nc.vector.tensor_copy(
                    wbs[:], pwb[:].rearrange("p (n m) -> p n m", n=NL)
                )
                # attnT[p, (hc, n)] = sum_m A[n, hc*128+p, m] * w[n, m]
                tmp = big.tile([128, HC, NL, M], BF16, tag="attmp")
                nc.vector.tensor_mul(
                    tmp[:],
                    at4[:],
                    wbs[:].unsqueeze(1).broadcast_to([128, HC, NL, M]),
                )
                atf = sml.tile([128, HC, NL], F32, tag="atf")
                nc.vector.tensor_reduce(
                    atf[:], tmp[:], axis=mybir.AxisListType.X, op=mybir.AluOpType.add
                )
                nc.vector.tensor_copy(aT_b[:], atf[:])

            # initial state: h = c0; hT, h-partials and attn for step 0
            transposes(h_sb)
            wd = attention_scores()
            pps_h = emit_h_rounds()
            attention_apply(wd)

            for t in range(steps):
                gi = gat.tile([NL, H], F32, tag="gi")
                gf = gat.tile([NL, H], F32, tag="gf")
                go = gat.tile([NL, H], F32, tag="go")
                gg = gat.tile([NL, H], F32, tag="gg")
                gates = [gi, gf, go, gg]
                for j in range(8):
                    xwt = xwp.tile([NL, 512], BF16, tag="xwt")
                    nc.sync.dma_start(
                        out=xwt[:],
                        in_=d_xws[t // 8][(t % 8) * NL : (t % 8 + 1) * NL,
                                          j * 512 : (j + 1) * 512],
                    )
                    jc = slice(j * 512, (j + 1) * 512)
                    pp = packp.tile([128, 512], F32, tag="pack_ps")
                    for r in range(2):
                        for g in range(4):
                            kk = 4 * r + g
                            nc.tensor.matmul(
                                pp[32 * g : 32 * g + 16, :], aT_b[:, kk],
                                wa[:, kk, jc],
                                start=(r == 0), stop=(r == 1),
                                tile_position=(0, 32 * g),
                                skip_group_check=True,
                            )
                    pps = gat.tile([128, 512], BF16, tag="pps")
                    nc.vector.tensor_copy(pps[:], pp[:])
                    pj = actp.tile([NL, 512], F32, tag="act_ps")
                    nc.tensor.matmul(pj[:], ssum[:], pps_h[j][:], start=True, stop=False)
                    nc.tensor.matmul(pj[:], ssum[:], pps[:], start=False, stop=False)
                    nc.tensor.matmul(
                        pj[:], id16b[:], xwt[:],
                        start=False, stop=True,
                    )
                    g = j // 2
                    half = (j % 2) * 512
                    func = (
                        mybir.ActivationFunctionType.Tanh
                        if g == 3
                        else mybir.ActivationFunctionType.Sigmoid
                    )
                    if "gates" not in _SKIP:
                        nc.scalar.activation(gates[g][:, half : half + 512], pj[:], func)

                # c = f*c + i*g ; h = o * tanh(c)
                if "elem" in _SKIP:
                    nc.sync.dma_start(out=d_y[:, t, :], in_=h_sb[:])
                    continue
                fc = gat.tile([NL, H], F32, tag="fc")
                nc.vector.tensor_mul(fc[:], gf[:], c_sb[:])
                ig = gat.tile([NL, H], F32, tag="ig")
                nc.vector.tensor_mul(ig[:], gi[:], gg[:])
                nc.vector.tensor_add(c_sb[:], fc[:], ig[:])
                th = gat.tile([NL, H], F32, tag="th")
                nc.scalar.activation(th[:], c_sb[:], mybir.ActivationFunctionType.Tanh)
                nc.vector.tensor_mul(h_sb[:], go[:], th[:])

                nc.sync.dma_start(out=d_y[:, t, :], in_=h_sb[:])

                if t < steps - 1:
                    transposes(h_sb)
                    wd = attention_scores()
                    pps_h = emit_h_rounds()
                    attention_apply(wd)

    nc.compile()
    _cache[key] = nc
    return nc


def _prepare(x, A, Wx, Wh, Wattn, b):
    x = np.ascontiguousarray(np.asarray(x, dtype=np.float32))
    A = np.ascontiguousarray(np.asarray(A, dtype=np.float32))
    Wxb = np.ascontiguousarray(np.asarray(Wx, dtype=np.float32).astype(np.float16))
    Whb = np.ascontiguousarray(np.asarray(Wh, dtype=np.float32).astype(np.float16))
    Wab = np.ascontiguousarray(np.asarray(Wattn, dtype=np.float32).astype(np.float16))
    b2 = np.ascontiguousarray(
        np.asarray(b, dtype=np.float32).reshape(1, K4).astype(np.float16)
    )

    ident = np.eye(16, dtype=np.float32)
    mask_mn = np.zeros((16, M * NL), dtype=np.float32)
    mask_nm = np.zeros((16, NL * M), dtype=np.float32)
    for a in range(M):
        for n in range(NL):
            mask_mn[n, a * NL + n] = 1.0  # [n, (m, n')]
            mask_nm[n, n * M + a] = 1.0   # [n', (n, m)]
    ones16 = np.ones((16, 128), dtype=np.float16)
    ssum = np.zeros((128, 16), dtype=np.float16)
    for g in range(4):
        for i in range(16):
            ssum[32 * g + i, i] = 1.0

    in_maps = []
    for k in range(NCORES):
        xs = x[k * NL : (k + 1) * NL]                     # [16, 64, 1024]
        xT = np.ascontiguousarray(
            xs.transpose(1, 0, 2).reshape(T * NL, D).T.astype(np.float16)
        )
        Ak = np.ascontiguousarray(A[k * NL : (k + 1) * NL].reshape(NL, H, M))
        in_maps.append(
            {
                "xT": xT,
                "A": Ak,
                "Wx": Wxb,
                "Wh": Whb,
                "Wa": Wab,
                "b": b2,
                "ident": ident,
                "mask_mn": mask_mn,
                "mask_nm": mask_nm,
                "ones16": ones16,
                "ssum": ssum,
            }
        )

    _cache["in_maps"] = in_maps
    return in_maps


def kernel(x, A, Wx, Wh, Wattn, b):
    nc = _build()
    in_maps = _prepare(x, A, Wx, Wh, Wattn, b)
    res = run_bass_kernel_spmd(nc, in_maps, core_ids=list(range(NCORES)))
    out = np.concatenate([res.results[k]["y"] for k in range(NCORES)], axis=0)
    return out.astype(np.float32)



# revision 47
# speedup vs baseline: 5.8304x; 5.8304x over previous
"""Attention-LSTM (CaptioningRNN) Trainium2 kernel.

Strategy: data-parallel over the batch N=128 across 8 NeuronCores (16
samples/core), zero cross-core communication.  All recurrent state lives
in a TRANSPOSED layout ([feature-partition, sample]) so that every
recurrent GEMM runs with the (SBUF-resident) weight tile as the PE
stationary operand and the narrow 16-sample state as the moving operand:

  act^T[f-tile, n] = xW^T (seeded via one full-bank identity matmul)
                   + sum_kc Wh[kc, f-tile]^T h2T[kc, n]
                   + sum_kc Wa[kc, f-tile]^T aT[kc, n]

Each gate owns its own PSUM bank, seeded by a start=True identity
matmul of the prefetched xW^T tile (full-bank write, so the zero-region
is cleared exactly once); all Wh/Wa matmuls then accumulate with
start=False, which lets the Wh segment run ahead of the attention chain
while each gate's Wa segment overlaps the previous gate's tanh.  The
seeds are emitted as step-boundary PE filler (they have no h2T
dependency), which keeps the tensor engine from idling - and its
p-state ramp from resetting - across the recurrence's serial point.

Sigmoid is reformulated as sigmoid(z) = 0.5*(1 + tanh(z/2)) with the
0.5 factors folded into pre-scaled weights and a doubled hidden state
(h2 = 2h), so the whole kernel only uses Exp/Tanh/Identity/Copy — all
members of one activation table (no in-loop ACT table reloads).

  2c' = (tf+1)*c + (ti+1)*g            (scalar_tensor_tensor + add)
  h2' = (to+1)*tanh(0.5 * 2c')         (= 2h', fixed up on the host)

Attention per step:
  scores: 8 wide matmuls -> [n', (m, n)], diagonal extracted on DVE via
          mask-mul + reduce; softmax normalized in [n, m] layout; the
          [m, n] transpose runs as one DVE 32x32 stream-transpose.
  apply:  128 one-column matmuls  attn^T[p,(hc,n)] = Am[m,hc,n,:]^T w[m,n]
          in halves so the aT copies pipeline under the last Wh segment.

Phase 1 precomputes xW^T = (x @ Wx + b)^T into DRAM scratch in the
transposed per-step layout the seeds consume.  The first half (t < 32)
runs before the loop with a deep Wx prefetch ring; the second half is
interleaved into the first steps of the recurrence as extra PE filler.
"""

import sys

sys.path.insert(0, "/opt/trn_rl_repo")

import numpy as np

import concourse.bass as bass  # noqa: F401
import concourse.mybir as mybir
import concourse.tile as tile
from concourse import bacc
from concourse.bass_utils import run_bass_kernel_spmd

F32 = mybir.dt.float32
BF16 = mybir.dt.float16  # IEEE fp16: same PE rate as bf16, more mantissa

N, T, D, H = 128, 64, 1024, 1024
K4 = 4 * H            # 4096
NCORES = 8
NL = N // NCORES      # 16 samples per core
M = 16                # spatial positions (4x4)
HC = H // 128         # 8 contraction chunks of h / attn
FC = K4 // 128        # 32 output f-tiles
SCALE = 1.0 / float(np.sqrt(H))

AF = mybir.ActivationFunctionType
OP = mybir.AluOpType
AX = mybir.AxisListType

_cache = {}


def _build(steps=T):
    key = ("nc", steps)
    if key in _cache:
        return _cache[key]

    nc = bacc.Bacc("TRN2", target_bir_lowering=False)

    # ---- kernel I/O ----------------------------------------------------
    # xT rows are D, columns are (t, n) so step t's block is contiguous.
    d_xT = nc.dram_tensor("xT", [D, T * NL], BF16, kind="ExternalInput")
    d_A = nc.dram_tensor("A", [NL, H, M], F32, kind="ExternalInput")
    # A^T in [m, hc, n, p] order for the attention-apply stationary tiles.
    d_atm = nc.dram_tensor("Am", [M, HC * NL * 128], BF16, kind="ExternalInput")
    d_wx = nc.dram_tensor("Wx", [D, K4], BF16, kind="ExternalInput")
    d_wh = nc.dram_tensor("Wh", [H, K4], BF16, kind="ExternalInput")
    d_wa = nc.dram_tensor("Wa", [H, K4], BF16, kind="ExternalInput")
    d_bT = nc.dram_tensor("bT", [128, FC], F32, kind="ExternalInput")
    d_mmn = nc.dram_tensor("mask_mn", [M, M * NL], F32, kind="ExternalInput")
    d_id = nc.dram_tensor("ident", [128, 128], BF16, kind="ExternalInput")
    d_id16f = nc.dram_tensor("id16f", [M, M], F32, kind="ExternalInput")
    d_yT = nc.dram_tensor("yT", [128, T, HC, NL], BF16, kind="ExternalOutput")

    # xW^T scratch, split by half so the first steps only wait on the
    # first-half writes.
    d_xws = [nc.dram_tensor(f"xws{b}", [128, T // 2, FC, NL], BF16) for b in range(2)]

    with tile.TileContext(nc) as tc:
      with (
          tc.tile_pool(name="wts", bufs=1) as wts,
          tc.tile_pool(name="stp", bufs=1) as stp,
      ):
        # Persistent weights / attention tensors (DMAs spread over three
        # queues so they overlap each other and phase-1 compute).
        wh = wts.tile([128, HC, K4], BF16, tag="wh")
        wa = wts.tile([128, HC, K4], BF16, tag="wa")
        wh_r = d_wh.rearrange("(kc p) f -> p kc f", p=128)
        wa_r = d_wa.rearrange("(kc p) f -> p kc f", p=128)
        wt_parts = [(wh, wh_r, kc) for kc in range(HC)] + [
            (wa, wa_r, kc) for kc in range(HC)
        ]
        atm = wts.tile([M, HC, NL, 128], BF16, tag="atm")
        mask_mn = wts.tile([M, M, NL], F32, tag="mask_mn")
        bT = wts.tile([128, FC], F32, tag="bT")
        nc.gpsimd.dma_start(out=bT[:], in_=d_bT[:])
        identb = wts.tile([128, 128], BF16, tag="identb")
        nc.gpsimd.dma_start(out=identb[:], in_=d_id[:])
        id16f = wts.tile([M, M], F32, tag="id16f")
        at4 = wts.tile([128, HC, NL, M], BF16, tag="at4")

        cT = stp.tile([128, HC, NL], F32, tag="cT")
        waste = stp.tile([128, 1], F32, tag="waste")
        # 32x32 pad buffer for the DVE stream-transpose of the softmax
        # weights (only the [0:16, 0:16] corner carries data)
        w16p = stp.tile([32, 32], BF16, tag="w16p")
        nc.vector.memset(w16p[:], 0.0)

        # ============== Phase 1: xW^T = (x @ Wx + b)^T ===================
        # blk 0 (t < 32) before the loop; blk 1 interleaved into the loop.
        p1s = tc.alloc_tile_pool(name="p1s", bufs=1)
        p1d = tc.alloc_tile_pool(name="p1d", bufs=2)
        p1p = tc.alloc_tile_pool(name="p1p", bufs=2, space="PSUM")

        xt = p1s.tile([128, HC, T * NL], BF16, tag="xt")
        nc.sync.dma_start(out=xt[:], in_=d_xT.rearrange("(kc p) r -> p kc r", p=128))
        wx_r = d_wx.rearrange("(kc p) f -> p kc f", p=128)

        # Wx tiles are fetched in PAIRS of fc per DMA (small transfers are
        # dominated by fixed DGE/sem latency); each pair tile has a single
        # linear owner and is released only after both halves are consumed.
        wxp_tiles = {}

        def fetch_wxp(fc0):
            wxp = p1d.tile([128, HC, 256], BF16, tag="wxp", bufs=2)
            q = nc.sync if (fc0 // 2) % 2 == 0 else nc.gpsimd
            q.dma_start(out=wxp[:], in_=wx_r[:, :, fc0 * 128 : (fc0 + 2) * 128])
            wxp_tiles[fc0] = wxp

        def emit_p1_half(blk, fc, ps, ob_ap):
            # one fc chunk: 8 accumulating matmuls + bias-copy to ob_ap
            wxp = wxp_tiles[fc - fc % 2]
            h = fc % 2
            for kc in range(HC):
                nc.tensor.matmul(
                    ps[:].rearrange("p t n -> p (t n)"),
                    wxp[:, kc, h * 128 : (h + 1) * 128],
                    xt[:, kc, blk * 512 : (blk + 1) * 512],
                    start=(kc == 0),
                    stop=(kc == HC - 1),
                )
            nc.vector.tensor_scalar_add(ob_ap, ps[:], bT[:, fc : fc + 1])

        # ---- phase-1a: blk 0, processed pair-at-a-time -----------------
        fetch_wxp(0)
        fetch_wxp(2)
        for j, fc0 in enumerate(range(0, FC, 2)):
            ob2 = p1d.tile([128, 32, 2, NL], BF16, tag="ob2", bufs=2)
            for h in range(2):
                ps = p1p.tile([128, 32, NL], F32, tag="p1ps")
                emit_p1_half(0, fc0 + h, ps, ob2[:, :, h, :])
            wxp_tiles.pop(fc0)
            if fc0 + 4 < FC:
                fetch_wxp(fc0 + 4)
            q = [nc.gpsimd, nc.sync, nc.scalar][j % 3]
            q.dma_start(out=d_xws[0][:, :, fc0 : fc0 + 2, :], in_=ob2[:])
            # trickle the recurrent-weight loads between pairs so their
            # transfers share DMA bandwidth fairly with the Wx stream
            if wt_parts:
                dst, src, kc = wt_parts.pop(0)
                nc.scalar.dma_start(out=dst[:, kc], in_=src[:, kc])
        while wt_parts:
            dst, src, kc = wt_parts.pop(0)
            nc.scalar.dma_start(out=dst[:, kc], in_=src[:, kc])

        # blk-1 chunk tasks interleaved into the loop: (fc, half) in order
        p1b = [(fc0 + h, h) for fc0 in range(0, FC, 2) for h in range(2)]

        def emit_p1b_task(t_unused):
            fc, h = p1b.pop(0)
            fc0 = fc - h
            if h == 0 and fc0 not in wxp_tiles:
                fetch_wxp(fc0)
            ps = p1p.tile([128, 32, NL], F32, tag="p1ps")
            ob = p1d.tile([128, 32, NL], BF16, tag="p1out", bufs=1)
            wxp = wxp_tiles[fc0]
            for kc in range(HC):
                nc.tensor.matmul(
                    ps[:].rearrange("p t n -> p (t n)"),
                    wxp[:, kc, h * 128 : (h + 1) * 128],
                    xt[:, kc, 512:1024],
                    start=(kc == 0),
                    stop=(kc == HC - 1),
                )
            # keep the loop's DVE chain clear: copy+bias on ACT
            nc.scalar.activation(
                ob[:].rearrange("p t n -> p (t n)"),
                ps[:].rearrange("p t n -> p (t n)"),
                AF.Identity,
                bias=bT[:, fc : fc + 1],
            )
            if h == 1:
                wxp_tiles.pop(fc0)
                if p1b and p1b[0][0] - p1b[0][1] not in wxp_tiles:
                    fetch_wxp(p1b[0][0] - p1b[0][1])
            q = [nc.gpsimd, nc.sync, nc.scalar][fc % 3]
            q.dma_start(out=d_xws[1][:, :, fc, :], in_=ob[:])

        # attention tensors: issued after the phase-1a chunk stream so
        # their transfers don't starve the Wx prefetch ring
        nc.gpsimd.dma_start(
            out=atm[:], in_=d_atm.rearrange("p (a b c) -> p a b c", a=HC, b=NL)
        )
        nc.gpsimd.dma_start(
            out=mask_mn[:], in_=d_mmn.rearrange("p (a c) -> p a c", a=M)
        )
        nc.gpsimd.dma_start(out=id16f[:], in_=d_id16f[:])

        # init state: c0 = mean_m A (transposed layout), h2_0 = 2*c0
        for hc in range(HC):
            a_stage = p1d.tile([128, NL, M], F32, tag="astage", bufs=1)
            nc.gpsimd.dma_start(
                out=a_stage[:],
                in_=d_A.rearrange("n (hc p) m -> hc p n m", hc=HC)[hc],
            )
            nc.vector.tensor_copy(at4[:, hc], a_stage[:])
            cacc = p1d.tile([128, NL], F32, tag="cacc")
            nc.vector.tensor_reduce(cacc[:], a_stage[:], axis=AX.X, op=OP.add)
            nc.vector.tensor_scalar_mul(cT[:, hc], cacc[:], 1.0 / M)

        # ============== Phase 2: recurrent loop =========================
        with (
            tc.tile_pool(name="xwp", bufs=2) as xwp,
            tc.tile_pool(name="gat", bufs=2) as gat,
            tc.tile_pool(name="sml", bufs=2) as sml,
            tc.tile_pool(name="psA", bufs=1, space="PSUM") as psA,
            tc.tile_pool(name="psF", bufs=1, space="PSUM") as psF,
            tc.tile_pool(name="psG", bufs=1, space="PSUM") as psG,
        ):
            h2T = stp.tile([128, HC, NL], BF16, tag="h2T", bufs=2)
            nc.vector.tensor_scalar_mul(h2T[:], cT[:], 2.0)

            xw_tiles = {}
            for tp in range(min(2, steps)):
                xw = xwp.tile([128, FC, NL], BF16, tag="xwt")
                nc.sync.dma_start(out=xw[:], in_=d_xws[tp // 32][:, tp % 32])
                xw_tiles[tp] = xw

            # act columns: [0:H]=i, [H:2H]=f, [2H:3H]=o, [3H:4H]=g.
            # Process f, i, g first (c' inputs), o last (h' tail).
            GATE_ORDER = [1, 0, 3, 2]

            def seed_pg(t):
                # Seed each gate's act^T bank with its xW^T + b slice in
                # one full-width matmul.  Emitted early (no h2T
                # dependency) to keep the PE busy through dependency
                # stalls.  Per-gate banks let each gate's Wa segment run
                # while the previous gate's tanh reads its own bank.
                xwt = xw_tiles.pop(t)
                pgs = {}
                for g4 in range(4):
                    pg = psG.tile([128, 8, NL], F32, tag=f"pg{g4}")
                    nc.tensor.matmul(
                        pg[:].rearrange("p a n -> p (a n)"),
                        identb[:],
                        xwt[:, g4 * 8 : (g4 + 1) * 8, :].rearrange(
                            "p a n -> p (a n)"
                        ),
                        start=True,
                        stop=False,
                        skip_group_check=True,
                    )
                    pgs[g4] = pg
                return pgs

            pgs_next = seed_pg(0) if steps else None

            for t in range(steps):
                if t + 2 < steps:
                    tp = t + 2
                    xw = xwp.tile([128, FC, NL], BF16, tag="xwt")
                    nc.sync.dma_start(out=xw[:], in_=d_xws[tp // 32][:, tp % 32])
                    xw_tiles[tp] = xw
                pgs = pgs_next

                # ---- attention scores: 8 wide matmuls ------------------
                # ps_sc[n', (m, n)] = sum_h h2[n', h] A[n, h, m]; the
                # diagonal n'==n is extracted on DVE via mask-mul+reduce.
                ps_sc = psA.tile([M, M, NL], F32, tag="ps_sc")
                for kc in range(HC):
                    nc.tensor.matmul(
                        ps_sc[:],
                        h2T[:, kc, :],
                        at4[:, kc].rearrange("p n m -> p m n"),
                        start=(kc == 0),
                        stop=(kc == HC - 1),
                        skip_group_check=True,
                    )
                smul = sml.tile([M, M, NL], F32, tag="smul", bufs=1)
                nc.vector.tensor_mul(smul[:], ps_sc[:], mask_mn[:])
                sc = sml.tile([M, M], F32, tag="sc")
                nc.vector.tensor_reduce(sc[:], smul[:], axis=AX.X, op=OP.add)
                # exp (scale folds the 1/sqrt(H) and the h2=2h factor)
                ex = sml.tile([M, M], BF16, tag="ex")
                nc.scalar.activation(ex[:], sc[:], AF.Exp, scale=SCALE * 0.5)
                sm = sml.tile([M, 1], F32, tag="sm")
                nc.vector.tensor_reduce(sm[:], ex[:], axis=AX.X, op=OP.add)
                rc = sml.tile([M, 1], F32, tag="rc")
                nc.vector.reciprocal(rc[:], sm[:])
                # normalized weights into the transpose pad, then one DVE
                # stream-transpose gives w^T without a PE/PSUM round-trip
                nc.vector.tensor_scalar_mul(w16p[0:M, 0:M], ex[:], rc[:])
                exT = sml.tile([32, 32], BF16, tag="exT")
                nc.vector.transpose(exT[:], w16p[:])
                wTn = exT[0:M, 0:M]

                def wh_segment(g4):
                    for fi in range(8):
                        f = g4 * 8 + fi
                        fsl = slice(f * 128, (f + 1) * 128)
                        for kc in range(HC):
                            nc.tensor.matmul(
                                pgs[g4][:, fi, :],
                                wh[:, kc, fsl],
                                h2T[:, kc, :],
                                start=False,
                                stop=False,
                                skip_group_check=True,
                            )

                wh_segment(GATE_ORDER[0])
                wh_segment(GATE_ORDER[1])
                wh_segment(GATE_ORDER[2])

                # ---- attention apply: 128 one-column matmuls, halved so
                # the aT copies pipeline under wh_o and the seeds --------
                pat = psF.tile([128, HC, NL], F32, tag="pat")
                aT = sml.tile([128, HC, NL], BF16, tag="aT")
                for half in range(2):
                    for hc in range(half * 4, half * 4 + 4):
                        for n in range(NL):
                            nc.tensor.matmul(
                                pat[:, hc, n : n + 1],
                                atm[:, hc, n, :],
                                wTn[:, n : n + 1],
                                start=True,
                                stop=True,
                                skip_group_check=True,
                            )
                    nc.vector.tensor_copy(
                        aT[:, half * 4 : half * 4 + 4, :],
                        pat[:, half * 4 : half * 4 + 4, :],
                    )

                wh_segment(GATE_ORDER[3])
                # next step's seeds: h2T-independent PE filler
                if t + 1 < steps:
                    pgs_next = seed_pg(t + 1)

                # ---- Wa segment (gate-ordered, kc-outer halves so the
                # first matmuls only need the first aT half) + state -----
                tg_tiles = {}
                u = v = th = None
                for g4 in GATE_ORDER:
                    pg = pgs[g4]
                    for kc in range(HC):
                        for fi in range(8):
                            f = g4 * 8 + fi
                            fsl = slice(f * 128, (f + 1) * 128)
                            nc.tensor.matmul(
                                pg[:, fi, :],
                                wa[:, kc, fsl],
                                aT[:, kc, :],
                                start=False,
                                stop=(kc == HC - 1) and (fi == 7),
                                skip_group_check=True,
                            )
                    tgate = gat.tile([128, 8, NL], F32, tag=f"tg{g4}", bufs=1)
                    nc.scalar.activation(tgate[:], pg[:], AF.Tanh)
                    tg_tiles[g4] = tgate

                    if g4 == 1:  # f ready: u = (tf+1)*c
                        u = gat.tile([128, HC, NL], F32, tag="u", bufs=1)
                        nc.vector.scalar_tensor_tensor(
                            u[:], tgate[:], 1.0, cT[:], OP.add, OP.mult
                        )
                    elif g4 == 3:  # g ready: v = (ti+1)*g ; 2c' = u+v
                        v = gat.tile([128, HC, NL], F32, tag="v", bufs=1)
                        nc.vector.scalar_tensor_tensor(
                            v[:], tg_tiles[0][:], 1.0, tgate[:], OP.add, OP.mult
                        )
                        c2 = gat.tile([128, HC, NL], F32, tag="c2", bufs=1)
                        nc.vector.tensor_add(c2[:], u[:], v[:])
                        # tanh(c') via the ACT input scale; the halved cell
                        # state itself is only needed next step (off-chain)
                        th = gat.tile([128, HC, NL], F32, tag="th", bufs=1)
                        nc.scalar.activation(th[:], c2[:], AF.Tanh, scale=0.5)
                        nc.vector.tensor_scalar_mul(cT[:], c2[:], 0.5)

                # tail filler: interleaved phase-1 chunk
                if p1b and steps == T:
                    emit_p1b_task(t)
                    if t < 4 and p1b:
                        emit_p1b_task(t)

                # o ready: h2' = (to+1)*tanh(c'), in hc-halves so the
                # next step's first score matmuls start half an op sooner
                h2T = stp.tile([128, HC, NL], BF16, tag="h2T", bufs=2)
                for hf in range(2):
                    sl = slice(hf * 4, hf * 4 + 4)
                    nc.vector.scalar_tensor_tensor(
                        h2T[:, sl], tg_tiles[2][:, sl], 1.0, th[:, sl],
                        OP.add, OP.mult,
                    )
                nc.gpsimd.dma_start(out=d_yT[:, t], in_=h2T[:])

        p1p.release()
        p1d.release()
        p1s.release()

    nc.compile()
    _cache[key] = nc
    return nc


def _prepare(x, A, Wx, Wh, Wattn, b):
    x = np.ascontiguousarray(np.asarray(x, dtype=np.float32))
    A = np.ascontiguousarray(np.asarray(A, dtype=np.float32))
    Wx = np.asarray(Wx, dtype=np.float32)
    Wh = np.asarray(Wh, dtype=np.float32)
    Wa = np.asarray(Wattn, dtype=np.float32)
    b = np.asarray(b, dtype=np.float32)

    # sigmoid(z) = 0.5*(1+tanh(z/2)): halve the i/f/o act columns; the
    # moving state is h2 = 2h, so Wh additionally gets a global 0.5.
    cs = np.ones((K4,), dtype=np.float32)
    cs[: 3 * H] = 0.5
    Wx_e = (Wx * cs).astype(np.float16)
    Wh_e = (Wh * (0.5 * cs)).astype(np.float16)
    Wa_e = (Wa * cs).astype(np.float16)
    b_e = b * cs
    bT = np.ascontiguousarray(b_e.reshape(FC, 128).T.astype(np.float32))

    mask_mn = np.zeros((M, M * NL), dtype=np.float32)
    for a in range(M):
        for n in range(NL):
            mask_mn[n, a * NL + n] = 1.0  # [n', (m, n)]
    ident = np.eye(128, dtype=np.float16)

    in_maps = []
    for k in range(NCORES):
        xs = x[k * NL : (k + 1) * NL]                     # [16, 64, 1024]
        xT = np.ascontiguousarray(
            xs.transpose(1, 0, 2).reshape(T * NL, D).T.astype(np.float16)
        )
        Ak = np.ascontiguousarray(A[k * NL : (k + 1) * NL].reshape(NL, H, M))
        Am = np.ascontiguousarray(
            Ak.reshape(NL, HC, 128, M)
            .transpose(3, 1, 0, 2)
            .reshape(M, HC * NL * 128)
            .astype(np.float16)
        )
        in_maps.append(
            {
                "xT": xT,
                "A": Ak,
                "Am": Am,
                "Wx": Wx_e,
                "Wh": Wh_e,
                "Wa": Wa_e,
                "bT": bT,
                "mask_mn": mask_mn,
                "ident": ident,
                "id16f": np.eye(M, dtype=np.float32),
            }
        )

    _cache["in_maps"] = in_maps
    return in_maps


def kernel(x, A, Wx, Wh, Wattn, b):
    nc = _build()
    in_maps = _prepare(x, A, Wx, Wh, Wattn, b)
    res = run_bass_kernel_spmd(nc, in_maps, core_ids=list(range(NCORES)))
    outs = []
    for k in range(NCORES):
        yT = np.asarray(res.results[k]["yT"], dtype=np.float32)  # [128,T,HC,NL]
        y = yT.transpose(3, 1, 2, 0).reshape(NL, T, H) * 0.5
        outs.append(y)
    return np.ascontiguousarray(np.concatenate(outs, axis=0).astype(np.float32))
